# revision 26
# baseline (speedup 1.0000x reference)
"""ChessBoardAttention Trainium2 kernel.

Math (per chessboard window of the input):
  x: [B=2, C=128, H=256, W=256] f32.  WS=8 chessboard phases.
  window (b, ph, pw) owns tokens (h, w) with h%8==ph, w%8==pw -> N=1024 tokens.
  q = x@Wq.T+bq [N,32]; k = x@Wk.T+bk [N,32]; v = x@Wv.T+bv [N,128]
  out = softmax(q k^T) v ; y = gamma*out + x

Sharding: 16 row-groups (b, ph), 2 per core. Each row-group holds the 8
pw-windows built from rows h==ph (mod 8) of batch b -> x[b,:,ph::8,:]
([128, 32, 256] slab, channel-partitioned). All compute for a window runs
on one core; no collectives.

Per-window on-chip pipeline (channel/token layouts chosen so softmax stats
are per-partition and the attention transpose rides the DMA XBAR):
  x_win  = stride-8 view of the slab: [c=128, t=1024]
  q^T,k^T = W^T.T @ x_win           (PE, bf16)   [32, 1024]
  v      = x_chunk.T @ Wv^T          (PE, bf16)   [m=128, c=128] per 128-token chunk
  S      = q_chunk.T @ k^T           (PE, bf16)   [n=128, m=1024] per n-chunk
  exp    = ACT Exp with accum_out -> Z[n] row sums
  attn   = exp * (gamma/Z[n])        (DVE, per-partition scalar)
  attn^T = DMA XBAR transpose        (SDMA, bf16)
  out^T  = v.T @ attn^T              (PE, accumulate over m-chunks) [c, 1024]
  y      = out^T + gamma*bv + x_win  (DVE scalar_tensor_tensor, in-place into slab)

softmax max-subtraction is dropped: scores are ~N(0, 0.3), exp is safe, and
softmax is shift-invariant so the result matches the reference.
"""

import sys

if "/opt/trn_rl_repo" not in sys.path:
    sys.path.insert(0, "/opt/trn_rl_repo")

from contextlib import ExitStack

import ml_dtypes
import numpy as np

import concourse.bacc as bacc
import concourse.bass as bass
import concourse.mybir as mybir
from concourse import bass_utils
from concourse.tile import TileContext

B, C, H, W = 2, 128, 256, 256
WS = 8
NH, NW = H // WS, W // WS  # 32, 32
N = NH * NW  # 1024 tokens per window
D = C // 4  # 32 q/k channels
NCORES = 8
PAIRS = 2  # (b, ph) row-groups per core
NCH = N // 128  # 8 chunks of 128 tokens
F32 = mybir.dt.float32
BF16 = mybir.dt.bfloat16

TRACE = False
LAST = {}

_CACHE = {}

def _emit(nc: bass.Bass):
    # xs is HOST-PERMUTED window-major: xs[g, c, pw, t] = x[b, c, (t//32)*8+ph, (t%32)*8+pw]
    xs = nc.dram_tensor("xs", [PAIRS, C, WS, N], F32, kind="ExternalInput").ap()
    wq = nc.dram_tensor("wq", [C, D], BF16, kind="ExternalInput").ap()
    wk = nc.dram_tensor("wk", [C, D], BF16, kind="ExternalInput").ap()
    wv = nc.dram_tensor("wv", [C, C], BF16, kind="ExternalInput").ap()
    bq = nc.dram_tensor("bq", [D, 1], F32, kind="ExternalInput").ap()
    bk = nc.dram_tensor("bk", [D, 1], F32, kind="ExternalInput").ap()
    gv = nc.dram_tensor("gv", [C, 1], F32, kind="ExternalInput").ap()  # gamma*bv
    gam = nc.dram_tensor("gam", [C, 1], F32, kind="ExternalInput").ap()  # gamma
    ys = nc.dram_tensor("ys", [PAIRS, C, WS, N], F32, kind="ExternalOutput").ap()

    with ExitStack() as ctx:
        tc = ctx.enter_context(TileContext(nc))
        consts = ctx.enter_context(tc.tile_pool(name="consts", bufs=1))
        xpool = ctx.enter_context(tc.tile_pool(name="xpool", bufs=2))
        xbpool = ctx.enter_context(tc.tile_pool(name="xbpool", bufs=2))
        qkpool = ctx.enter_context(tc.tile_pool(name="qkpool", bufs=2))
        vpool = ctx.enter_context(tc.tile_pool(name="vpool", bufs=2))
        epool = ctx.enter_context(tc.tile_pool(name="epool", bufs=3))
        apool = ctx.enter_context(tc.tile_pool(name="apool", bufs=2))
        # bufs=16: one slot per window -> no slot reuse -> no WAR waits on the
        # ACT exp instructions (TensorScalarPtr/ACTIVATE have few wait slots)
        zpool = ctx.enter_context(tc.tile_pool(name="zpool", bufs=16))
        ps_s = ctx.enter_context(tc.tile_pool(name="ps_s", bufs=2, space="PSUM"))
        ps_o = ctx.enter_context(tc.tile_pool(name="ps_o", bufs=1, space="PSUM"))
        ps_m = ctx.enter_context(tc.tile_pool(name="ps_m", bufs=1, space="PSUM"))

        wq_sb = consts.tile([C, D], BF16)
        nc.sync.dma_start(out=wq_sb, in_=wq)
        wk_sb = consts.tile([C, D], BF16)
        nc.sync.dma_start(out=wk_sb, in_=wk)
        wv_sb = consts.tile([C, C], BF16)
        nc.sync.dma_start(out=wv_sb, in_=wv)
        bq_sb = consts.tile([D, 1], F32)
        nc.sync.dma_start(out=bq_sb, in_=bq)
        bk_sb = consts.tile([D, 1], F32)
        nc.sync.dma_start(out=bk_sb, in_=bk)
        gv_sb = consts.tile([C, 1], F32)
        nc.sync.dma_start(out=gv_sb, in_=gv)
        gam_sb = consts.tile([C, 1], F32)
        nc.sync.dma_start(out=gam_sb, in_=gam)

        # Touch every const once on DVE so later DVE ops (esp. TensorScalarPtr,
        # which walrus limits to ONE sync wait) never carry const-DMA waits.
        scratch = consts.tile([C, 8], F32)
        for i, t in enumerate([wq_sb, wk_sb, wv_sb, bq_sb, bk_sb, gv_sb, gam_sb]):
            nc.vector.tensor_copy(out=scratch[: t.shape[0], i : i + 1], in_=t[:, 0:1])

        for g in range(PAIRS):
            x_slab = xpool.tile([C, WS, N], F32)
            nc.gpsimd.dma_start(out=x_slab, in_=xs[g])
            xb2 = xbpool.tile([C, WS, N], BF16)
            nc.vector.tensor_copy(out=xb2, in_=x_slab)  # contiguous cast, 2x mode

            for pw in range(WS):
                xw = xb2[:, pw, :]  # [128, 1024] bf16, contiguous
                xw_f32 = x_slab[:, pw, :]  # [128, 1024] f32, contiguous

                # ---- q/k projections: [32, 1024] = W^T.T @ x_win ----
                pqk = ps_m.tile([C, N], F32, tag="mm")
                for h in range(2):
                    nc.tensor.matmul(
                        pqk[:D, bass.ts(h, 512)], wq_sb, xw[:, bass.ts(h, 512)]
                    )
                q_sb = qkpool.tile([D, N], BF16, tag="q")
                nc.vector.tensor_scalar_add(out=q_sb, in0=pqk[:D, :], scalar1=bq_sb)
                pqk2 = ps_m.tile([C, N], F32, tag="mm")
                for h in range(2):
                    nc.tensor.matmul(
                        pqk2[:D, bass.ts(h, 512)], wk_sb, xw[:, bass.ts(h, 512)]
                    )
                k_sb = qkpool.tile([D, N], BF16, tag="k")
                nc.vector.tensor_scalar_add(out=k_sb, in0=pqk2[:D, :], scalar1=bk_sb)

                # ---- v: [m, c] = x_chunk.T @ Wv^T per 128-token chunk ----
                pv = ps_m.tile([C, N], F32, tag="mm")
                for mc in range(NCH):
                    nc.tensor.matmul(
                        pv[:, bass.ts(mc, 128)], xw[:, bass.ts(mc, 128)], wv_sb
                    )
                v_sb = vpool.tile([C, N], BF16)
                nc.vector.tensor_copy(out=v_sb, in_=pv)

                # ---- scores + softmax + attn^T + AV, pipelined per half ----
                e_sb = epool.tile([128, NCH, N], BF16)
                z = zpool.tile([128, NCH], F32, tag="z")
                izg = zpool.tile([128, NCH], F32, tag="izg")
                po = ps_o.tile([C, N], F32)
                for hh in range(2):
                    for ncc in range(hh * 4, hh * 4 + 4):
                        ps = ps_s.tile([128, N], F32)
                        for h in range(2):
                            nc.tensor.matmul(
                                ps[:, bass.ts(h, 512)],
                                q_sb[:, bass.ts(ncc, 128)],
                                k_sb[:, bass.ts(h, 512)],
                            )
                        nc.scalar.activation(
                            out=e_sb[:, ncc, :],
                            in_=ps,
                            func=mybir.ActivationFunctionType.Exp,
                            accum_out=z[:, ncc : ncc + 1],
                        )
                    hs = slice(hh * 4, hh * 4 + 4)
                    nc.vector.reciprocal(out=izg[:, hs], in_=z[:, hs])
                    nc.vector.tensor_scalar_mul(
                        out=izg[:, hs], in0=izg[:, hs], scalar1=gam_sb
                    )
                    for ncc in range(hh * 4, hh * 4 + 4):
                        nc.vector.tensor_scalar_mul(
                            out=e_sb[:, ncc, :],
                            in0=e_sb[:, ncc, :],
                            scalar1=izg[:, ncc : ncc + 1],
                        )
                    # ---- attn^T for this n-half via one DMA XBAR transpose ----
                    # in [128, 4096]: f = nccL*1024 + m; out[p, d1, j] =
                    # in[j, d1*128+p], d1 = nccL*8+mc -> at_h[p, nccL*8+mc, j]
                    # = attn[(hh*4+nccL)*128+j, mc*128+p]. Separate tile per
                    # half so AV(h) only depends on its own transpose.
                    at_h = apool.tile([128, NCH * 4, 128], BF16, tag=f"at{hh}")
                    nc.sync.dma_start(
                        out=at_h,
                        in_=e_sb[:, hs, :].rearrange("p a m -> p (a m)"),
                        transpose=True,
                    )
                    at_r = at_h.rearrange("p (a b) j -> p b a j", b=NCH)

                    # ---- out^T[c, n-half] = sum_m v[m, c] * attn^T[m, n] ----
                    for mc in range(NCH):
                        nc.tensor.matmul(
                            po[:, bass.ts(hh, 512)],
                            v_sb[:, bass.ts(mc, 128)],
                            at_r[:, mc, :, :],
                            start=(mc == 0),
                            stop=(mc == NCH - 1),
                        )

                # ---- epilogue: y = out^T + gamma*bv + x (in-place into slab) ----
                nc.vector.scalar_tensor_tensor(
                    out=xw_f32,
                    in0=po,
                    scalar=gv_sb,
                    in1=xw_f32,
                    op0=mybir.AluOpType.add,
                    op1=mybir.AluOpType.add,
                )

            nc.gpsimd.dma_start(out=ys[g], in_=x_slab)
    return nc


def _get_nc():
    if "nc" not in _CACHE:
        nc = bacc.Bacc(
            "TRN2",
            target_bir_lowering=False,
            debug=False,
            enable_asserts=False,
            num_devices=NCORES,
        )
        _emit(nc)
        # bacc passes: split multi-sem waits into EventSemaphores (HW allows
        # one wait per instruction), move matmul waits to ldweights, etc.
        nc.finalize()
        _CACHE["nc"] = nc
    return _CACHE["nc"]


def _shard_inputs(x, Wq, bq, Wk, bk, Wv, bv, gamma):
    bf = ml_dtypes.bfloat16
    x = np.ascontiguousarray(np.asarray(x, np.float32))
    wq_h = np.ascontiguousarray(np.asarray(Wq, np.float32).T).astype(bf)
    wk_h = np.ascontiguousarray(np.asarray(Wk, np.float32).T).astype(bf)
    wv_h = np.ascontiguousarray(np.asarray(Wv, np.float32).T).astype(bf)
    bq_h = np.ascontiguousarray(np.asarray(bq, np.float32).reshape(D, 1))
    bk_h = np.ascontiguousarray(np.asarray(bk, np.float32).reshape(D, 1))
    g = float(np.asarray(gamma, np.float32).reshape(-1)[0])
    gv_h = np.ascontiguousarray((g * np.asarray(bv, np.float32)).reshape(C, 1))
    gam_h = np.full((C, 1), g, np.float32)
    # window-major permute: x6[b, c, i, ph, j, pw] -> slab[c, pw, i*32+j]
    x6 = x.reshape(B, C, NH, WS, NW, WS)
    in_maps = []
    for core in range(NCORES):
        slabs = np.stack(
            [
                np.ascontiguousarray(
                    x6[(PAIRS * core + j) // WS, :, :, (PAIRS * core + j) % WS, :, :]
                    .transpose(0, 3, 1, 2)  # [c, pw, i, j]
                    .reshape(C, WS, N)
                )
                for j in range(PAIRS)
            ]
        )
        in_maps.append(
            dict(
                xs=slabs,
                wq=wq_h,
                wk=wk_h,
                wv=wv_h,
                bq=bq_h,
                bk=bk_h,
                gv=gv_h,
                gam=gam_h,
            )
        )
    return in_maps


def kernel(x, Wq, bq, Wk, bk, Wv, bv, gamma):
    nc = _get_nc()
    in_maps = _shard_inputs(x, Wq, bq, Wk, bk, Wv, bv, gamma)
    res = bass_utils.run_bass_kernel_spmd(
        nc, in_maps, core_ids=list(range(NCORES)), trace=TRACE
    )
    LAST["exec_time_ns"] = res.exec_time_ns
    LAST["results"] = res
    y = np.empty((B, C, H, W), np.float32)
    y6 = y.reshape(B, C, NH, WS, NW, WS)
    for core in range(NCORES):
        out = res.results[core]["ys"]  # [PAIRS, C, WS, N]
        for j in range(PAIRS):
            p = PAIRS * core + j
            # [c, pw, i, j] -> [c, i, j, pw]
            y6[p // WS, :, :, p % WS, :, :] = (
                out[j].reshape(C, WS, NH, NW).transpose(0, 2, 3, 1)
            )
    return y


# revision 28
# speedup vs baseline: 1.0111x; 1.0111x over previous
"""ChessBoardAttention Trainium2 kernel.

Math (per chessboard window of the input):
  x: [B=2, C=128, H=256, W=256] f32.  WS=8 chessboard phases.
  window (b, ph, pw) owns tokens (h, w) with h%8==ph, w%8==pw -> N=1024 tokens.
  q = x@Wq.T+bq [N,32]; k = x@Wk.T+bk [N,32]; v = x@Wv.T+bv [N,128]
  out = softmax(q k^T) v ; y = gamma*out + x

Sharding: 16 row-groups (b, ph), 2 per core. Each row-group holds the 8
pw-windows built from rows h==ph (mod 8) of batch b -> x[b,:,ph::8,:]
([128, 32, 256] slab, channel-partitioned). All compute for a window runs
on one core; no collectives.

Per-window on-chip pipeline (channel/token layouts chosen so softmax stats
are per-partition and the attention transpose rides the DMA XBAR):
  x_win  = stride-8 view of the slab: [c=128, t=1024]
  q^T,k^T = W^T.T @ x_win           (PE, bf16)   [32, 1024]
  v      = x_chunk.T @ Wv^T          (PE, bf16)   [m=128, c=128] per 128-token chunk
  S      = q_chunk.T @ k^T           (PE, bf16)   [n=128, m=1024] per n-chunk
  exp    = ACT Exp with accum_out -> Z[n] row sums
  attn   = exp * (gamma/Z[n])        (DVE, per-partition scalar)
  attn^T = DMA XBAR transpose        (SDMA, bf16)
  out^T  = v.T @ attn^T              (PE, accumulate over m-chunks) [c, 1024]
  y      = out^T + gamma*bv + x_win  (DVE scalar_tensor_tensor, in-place into slab)

softmax max-subtraction is dropped: scores are ~N(0, 0.3), exp is safe, and
softmax is shift-invariant so the result matches the reference.
"""

import sys

if "/opt/trn_rl_repo" not in sys.path:
    sys.path.insert(0, "/opt/trn_rl_repo")

from contextlib import ExitStack

import ml_dtypes
import numpy as np

import concourse.bacc as bacc
import concourse.bass as bass
import concourse.mybir as mybir
from concourse import bass_utils
from concourse.tile import TileContext

B, C, H, W = 2, 128, 256, 256
WS = 8
NH, NW = H // WS, W // WS  # 32, 32
N = NH * NW  # 1024 tokens per window
D = C // 4  # 32 q/k channels
NCORES = 8
PAIRS = 2  # (b, ph) row-groups per core
NCH = N // 128  # 8 chunks of 128 tokens
F32 = mybir.dt.float32
BF16 = mybir.dt.bfloat16

TRACE = False
LAST = {}

_CACHE = {}

def _emit(nc: bass.Bass):
    # xs is HOST-PERMUTED window-major: xs[g, c, pw, t] = x[b, c, (t//32)*8+ph, (t%32)*8+pw]
    xs = nc.dram_tensor("xs", [PAIRS, C, WS, N], F32, kind="ExternalInput").ap()
    wq = nc.dram_tensor("wq", [C, D], BF16, kind="ExternalInput").ap()
    wk = nc.dram_tensor("wk", [C, D], BF16, kind="ExternalInput").ap()
    wv = nc.dram_tensor("wv", [C, C], BF16, kind="ExternalInput").ap()
    bq = nc.dram_tensor("bq", [D, 1], F32, kind="ExternalInput").ap()
    bk = nc.dram_tensor("bk", [D, 1], F32, kind="ExternalInput").ap()
    gv = nc.dram_tensor("gv", [C, 1], F32, kind="ExternalInput").ap()  # gamma*bv
    gam = nc.dram_tensor("gam", [C, 1], F32, kind="ExternalInput").ap()  # gamma
    ys = nc.dram_tensor("ys", [PAIRS, C, WS, N], F32, kind="ExternalOutput").ap()

    with ExitStack() as ctx:
        tc = ctx.enter_context(TileContext(nc))
        consts = ctx.enter_context(tc.tile_pool(name="consts", bufs=1))
        xpool = ctx.enter_context(tc.tile_pool(name="xpool", bufs=2))
        xbpool = ctx.enter_context(tc.tile_pool(name="xbpool", bufs=2))
        qkpool = ctx.enter_context(tc.tile_pool(name="qkpool", bufs=2))
        vpool = ctx.enter_context(tc.tile_pool(name="vpool", bufs=2))
        epool = ctx.enter_context(tc.tile_pool(name="epool", bufs=3))
        apool = ctx.enter_context(tc.tile_pool(name="apool", bufs=2))
        # bufs=16: one slot per window -> no slot reuse -> no WAR waits on the
        # ACT exp instructions (TensorScalarPtr/ACTIVATE have few wait slots)
        zpool = ctx.enter_context(tc.tile_pool(name="zpool", bufs=16))
        ps_s = ctx.enter_context(tc.tile_pool(name="ps_s", bufs=2, space="PSUM"))
        ps_o = ctx.enter_context(tc.tile_pool(name="ps_o", bufs=1, space="PSUM"))
        ps_m = ctx.enter_context(tc.tile_pool(name="ps_m", bufs=1, space="PSUM"))

        wq_sb = consts.tile([C, D], BF16)
        nc.sync.dma_start(out=wq_sb, in_=wq)
        wk_sb = consts.tile([C, D], BF16)
        nc.sync.dma_start(out=wk_sb, in_=wk)
        wv_sb = consts.tile([C, C], BF16)
        nc.sync.dma_start(out=wv_sb, in_=wv)
        bq_sb = consts.tile([D, 1], F32)
        nc.sync.dma_start(out=bq_sb, in_=bq)
        bk_sb = consts.tile([D, 1], F32)
        nc.sync.dma_start(out=bk_sb, in_=bk)
        gv_sb = consts.tile([C, 1], F32)
        nc.sync.dma_start(out=gv_sb, in_=gv)
        gam_sb = consts.tile([C, 1], F32)
        nc.sync.dma_start(out=gam_sb, in_=gam)

        # Touch every const once on DVE so later DVE ops (esp. TensorScalarPtr,
        # which walrus limits to ONE sync wait) never carry const-DMA waits.
        scratch = consts.tile([C, 8], F32)
        for i, t in enumerate([wq_sb, wk_sb, wv_sb, bq_sb, bk_sb, gv_sb, gam_sb]):
            nc.vector.tensor_copy(out=scratch[: t.shape[0], i : i + 1], in_=t[:, 0:1])

        for g in range(PAIRS):
            x_slab = xpool.tile([C, WS, N], F32)
            nc.gpsimd.dma_start(out=x_slab, in_=xs[g])
            xb2 = xbpool.tile([C, WS, N], BF16)
            nc.vector.tensor_copy(out=xb2, in_=x_slab)  # contiguous cast, 2x mode

            for pw in range(WS):
                xw = xb2[:, pw, :]  # [128, 1024] bf16, contiguous
                xw_f32 = x_slab[:, pw, :]  # [128, 1024] f32, contiguous

                # ---- q/k projections: [32, 1024] = W^T.T @ x_win ----
                pqk = ps_m.tile([C, N], F32, tag="mm")
                for h in range(2):
                    nc.tensor.matmul(
                        pqk[:D, bass.ts(h, 512)], wq_sb, xw[:, bass.ts(h, 512)]
                    )
                q_sb = qkpool.tile([D, N], BF16, tag="q")
                nc.vector.tensor_scalar_add(out=q_sb, in0=pqk[:D, :], scalar1=bq_sb)
                pqk2 = ps_m.tile([C, N], F32, tag="mm")
                for h in range(2):
                    nc.tensor.matmul(
                        pqk2[:D, bass.ts(h, 512)], wk_sb, xw[:, bass.ts(h, 512)]
                    )
                k_sb = qkpool.tile([D, N], BF16, tag="k")
                nc.vector.tensor_scalar_add(out=k_sb, in0=pqk2[:D, :], scalar1=bk_sb)

                # ---- v^T[c_out, m] = Wv @ x_win (1 stationary, 2 wide mms),
                # then v[m, c] chunks via DMA XBAR instead of 8 PE matmuls ----
                pv = ps_m.tile([C, N], F32, tag="mm")
                for h in range(2):
                    nc.tensor.matmul(
                        pv[:, bass.ts(h, 512)], wv_sb, xw[:, bass.ts(h, 512)]
                    )
                v_sb = vpool.tile([C, N], BF16)
                nc.vector.tensor_copy(out=v_sb, in_=pv)
                # vt[p, mc, j] = v_sb[j, mc*128+p] = v[m=mc*128+p, c=j]
                vt = vpool.tile([128, NCH, 128], BF16, tag="vt")
                nc.sync.dma_start(out=vt, in_=v_sb, transpose=True)

                # ---- scores + softmax + attn^T + AV, pipelined per half ----
                e_sb = epool.tile([128, NCH, N], BF16)
                z = zpool.tile([128, NCH], F32, tag="z")
                izg = zpool.tile([128, NCH], F32, tag="izg")
                po = ps_o.tile([C, N], F32)
                for hh in range(2):
                    for ncc in range(hh * 4, hh * 4 + 4):
                        ps = ps_s.tile([128, N], F32)
                        for h in range(2):
                            nc.tensor.matmul(
                                ps[:, bass.ts(h, 512)],
                                q_sb[:, bass.ts(ncc, 128)],
                                k_sb[:, bass.ts(h, 512)],
                            )
                        nc.scalar.activation(
                            out=e_sb[:, ncc, :],
                            in_=ps,
                            func=mybir.ActivationFunctionType.Exp,
                            accum_out=z[:, ncc : ncc + 1],
                        )
                    hs = slice(hh * 4, hh * 4 + 4)
                    nc.vector.reciprocal(out=izg[:, hs], in_=z[:, hs])
                    nc.vector.tensor_scalar_mul(
                        out=izg[:, hs], in0=izg[:, hs], scalar1=gam_sb
                    )
                    for ncc in range(hh * 4, hh * 4 + 4):
                        nc.vector.tensor_scalar_mul(
                            out=e_sb[:, ncc, :],
                            in0=e_sb[:, ncc, :],
                            scalar1=izg[:, ncc : ncc + 1],
                        )
                    # ---- attn^T for this n-half via one DMA XBAR transpose ----
                    # in [128, 4096]: f = nccL*1024 + m; out[p, d1, j] =
                    # in[j, d1*128+p], d1 = nccL*8+mc -> at_h[p, nccL*8+mc, j]
                    # = attn[(hh*4+nccL)*128+j, mc*128+p]. Separate tile per
                    # half so AV(h) only depends on its own transpose.
                    at_h = apool.tile([128, NCH * 4, 128], BF16, tag=f"at{hh}")
                    nc.sync.dma_start(
                        out=at_h,
                        in_=e_sb[:, hs, :].rearrange("p a m -> p (a m)"),
                        transpose=True,
                    )
                    at_r = at_h.rearrange("p (a b) j -> p b a j", b=NCH)

                    # ---- out^T[c, n-half] = sum_m v[m, c] * attn^T[m, n] ----
                    for mc in range(NCH):
                        nc.tensor.matmul(
                            po[:, bass.ts(hh, 512)],
                            vt[:, mc, :],
                            at_r[:, mc, :, :],
                            start=(mc == 0),
                            stop=(mc == NCH - 1),
                        )

                # ---- epilogue: y = out^T + gamma*bv + x (in-place into slab) ----
                nc.vector.scalar_tensor_tensor(
                    out=xw_f32,
                    in0=po,
                    scalar=gv_sb,
                    in1=xw_f32,
                    op0=mybir.AluOpType.add,
                    op1=mybir.AluOpType.add,
                )

            nc.gpsimd.dma_start(out=ys[g], in_=x_slab)
    return nc


def _get_nc():
    if "nc" not in _CACHE:
        nc = bacc.Bacc(
            "TRN2",
            target_bir_lowering=False,
            debug=False,
            enable_asserts=False,
            num_devices=NCORES,
        )
        _emit(nc)
        # bacc passes: split multi-sem waits into EventSemaphores (HW allows
        # one wait per instruction), move matmul waits to ldweights, etc.
        nc.finalize()
        _CACHE["nc"] = nc
    return _CACHE["nc"]


def _shard_inputs(x, Wq, bq, Wk, bk, Wv, bv, gamma):
    bf = ml_dtypes.bfloat16
    x = np.ascontiguousarray(np.asarray(x, np.float32))
    wq_h = np.ascontiguousarray(np.asarray(Wq, np.float32).T).astype(bf)
    wk_h = np.ascontiguousarray(np.asarray(Wk, np.float32).T).astype(bf)
    wv_h = np.ascontiguousarray(np.asarray(Wv, np.float32).T).astype(bf)
    bq_h = np.ascontiguousarray(np.asarray(bq, np.float32).reshape(D, 1))
    bk_h = np.ascontiguousarray(np.asarray(bk, np.float32).reshape(D, 1))
    g = float(np.asarray(gamma, np.float32).reshape(-1)[0])
    gv_h = np.ascontiguousarray((g * np.asarray(bv, np.float32)).reshape(C, 1))
    gam_h = np.full((C, 1), g, np.float32)
    # window-major permute: x6[b, c, i, ph, j, pw] -> slab[c, pw, i*32+j]
    x6 = x.reshape(B, C, NH, WS, NW, WS)
    in_maps = []
    for core in range(NCORES):
        slabs = np.stack(
            [
                np.ascontiguousarray(
                    x6[(PAIRS * core + j) // WS, :, :, (PAIRS * core + j) % WS, :, :]
                    .transpose(0, 3, 1, 2)  # [c, pw, i, j]
                    .reshape(C, WS, N)
                )
                for j in range(PAIRS)
            ]
        )
        in_maps.append(
            dict(
                xs=slabs,
                wq=wq_h,
                wk=wk_h,
                wv=wv_h,
                bq=bq_h,
                bk=bk_h,
                gv=gv_h,
                gam=gam_h,
            )
        )
    return in_maps


def kernel(x, Wq, bq, Wk, bk, Wv, bv, gamma):
    nc = _get_nc()
    in_maps = _shard_inputs(x, Wq, bq, Wk, bk, Wv, bv, gamma)
    res = bass_utils.run_bass_kernel_spmd(
        nc, in_maps, core_ids=list(range(NCORES)), trace=TRACE
    )
    LAST["exec_time_ns"] = res.exec_time_ns
    LAST["results"] = res
    y = np.empty((B, C, H, W), np.float32)
    y6 = y.reshape(B, C, NH, WS, NW, WS)
    for core in range(NCORES):
        out = res.results[core]["ys"]  # [PAIRS, C, WS, N]
        for j in range(PAIRS):
            p = PAIRS * core + j
            # [c, pw, i, j] -> [c, i, j, pw]
            y6[p // WS, :, :, p % WS, :, :] = (
                out[j].reshape(C, WS, NH, NW).transpose(0, 2, 3, 1)
            )
    return y
